# revision 8
# baseline (speedup 1.0000x reference)
"""Masked dot-product attention on 8 Trainium2 NeuronCores.

Problem: q,k,v [16, 2048, 128] fp32, valid_len [16] int -> out [16, 2048, 128].
out[b] = softmax(mask(q[b] @ k[b].T / sqrt(128), valid_len[b])) @ v[b]

Strategy (v2, ~2-3x over the batch-parallel flash baseline):
  - Keys beyond valid_len contribute exp(-inf)=0 exactly, so fully-masked
    128-key tiles can be skipped. For the graded input only ~127 of 256
    (batch, key-tile) pairs have any valid keys.
  - The work unit is a (batch, key-tile) pair. A single SPMD program runs
    S fixed-size "segments" per core (sizes compile-time, e.g. (10,5,3));
    the host binds each (core, segment) slot to any batch + tile range and
    sums the partial results: each segment emits an UNNORMALIZED O^T
    [d,2048] plus per-query exp-sums; host adds partials per batch and
    divides. Unused slot capacity is padded with zero tiles + -inf bias.
  - Mask is applied as the ACT per-partition bias: st layout is
    [key partition, query free], so a [128,1] bias of 0/-30000 per key
    tile masks invalid keys inside the exp instruction. No mask matmul
    stream (1/3 of baseline PE work), no zeroed-V copy, no mb tensor.
  - Per segment, per 1024-query pass: score matmul kt_t.T @ qt -> st
    (PSUM, f32r), ACT exp(st*scale+bias) -> pt (SBUF, bf16), PV matmul
    vs_t.T @ pt accumulating OT in PSUM across the tile loop, DVE
    acc += pt (bf16) accumulates the softmax denominators; acc tiles are
    DMA'd out raw and partition-summed on host.
  - OT is drained PSUM->SBUF (cast bf16) on the otherwise-idle GPSIMD
    engine, then DMA'd. PE runs only the two irreducible matmul streams.
"""

import os
from collections import deque

import numpy as np

import concourse.tile as tile
from concourse import bacc, mybir
from concourse.bass_utils import run_bass_kernel_spmd

B, SQ, SK, D = 16, 2048, 2048, 128
NCORES = 8
P = 128
QW = 1024  # query pass width (PSUM: OT [128,1024] fp32 = 2 banks)
NQP = SQ // QW  # query passes
SCALE = 1.0 / float(np.sqrt(D))
NEG_BIAS = -30000.0

FP32 = mybir.dt.float32
F32R = mybir.dt.float32r
BF16 = mybir.dt.bfloat16

# ---------------------------------------------------------------------------
# packing: cover per-batch tile counts with 8 copies of each segment size
# ---------------------------------------------------------------------------

# candidate segment-size tuples, tried in order (first feasible wins)
_CANDIDATES = [
    (10, 5, 3),
    (10, 6, 3),
    (11, 5, 3),
    (11, 6, 4),
    (12, 6, 4),
    (13, 7, 4),
    (14, 7, 5),
    (16, 8, 8),  # always feasible for 16 batches of <=16 tiles
]


def _try_pack(nt, sizes, rng):
    """Try to cover tile counts nt (list of (batch, count)) with 8 slots of
    each size in `sizes`. Returns list of chunks (batch, tile_start, n, size)
    or None."""
    slots = []
    for sz in sizes:
        slots += [sz] * 8
    slots.sort(reverse=True)
    order = sorted(range(len(nt)), key=lambda i: -nt[i][1])
    if rng is not None:
        order = list(order)
        rng.shuffle(order)
    avail = list(slots)
    chunks = []
    for i in order:
        b, cnt = nt[i]
        t0 = 0
        rem = cnt
        while rem > 0:
            # if some slot can finish the batch, take the smallest such;
            # otherwise take the largest slot and continue
            ge = [s for s in avail if s >= rem]
            if ge:
                s = min(ge)
                if rng is not None and len(ge) > 1 and rng.rand() < 0.3:
                    s = rng.choice(ge)
            else:
                if not avail:
                    return None
                s = max(avail)
            avail.remove(s)
            take = min(s, rem)
            chunks.append((b, t0, take, s))
            t0 += take
            rem -= take
    return chunks


def _plan(valid_len):
    """Choose segment sizes + assignment of chunks to (core, seg) slots."""
    nt = []
    for b in range(B):
        c = int(min(SK, max(0, int(valid_len[b]))))
        c = (c + P - 1) // P
        if c > 0:
            nt.append((b, c))
    rng = np.random.RandomState(0)
    for sizes in _CANDIDATES:
        if sum(sizes) * 8 < sum(c for _, c in nt):
            continue
        for trial in range(64):
            chunks = _try_pack(nt, sizes, None if trial == 0 else rng)
            if chunks is not None:
                # map chunks to (core, seg) slots: slot list per size
                free = {}
                for s_idx, sz in enumerate(sizes):
                    free.setdefault(sz, [])
                    free[sz] += [(c, s_idx) for c in range(NCORES)]
                assign = {}  # (core, seg) -> (batch, t0, n)
                ok = True
                for b, t0, n, sz in chunks:
                    if not free[sz]:
                        ok = False
                        break
                    core, s_idx = free[sz].pop()
                    assign[(core, s_idx)] = (b, t0, n)
                if ok:
                    return sizes, assign
    raise RuntimeError("packing failed")


# ---------------------------------------------------------------------------
# device kernel (one per segment-size tuple, cached)
# ---------------------------------------------------------------------------


def _build_kernel(ctx, tc, outs, ins, sizes):
    nc = tc.nc
    TOT = sum(sizes)
    big = ctx.enter_context(tc.tile_pool(name="big", bufs=1))
    ptp = ctx.enter_context(tc.tile_pool(name="ptp", bufs=4))
    accp = ctx.enter_context(tc.tile_pool(name="accp", bufs=2))
    osb = ctx.enter_context(tc.tile_pool(name="osb", bufs=2))
    stp = ctx.enter_context(tc.tile_pool(name="stp", bufs=2, space="PSUM"))
    otp = ctx.enter_context(tc.tile_pool(name="otp", bufs=2, space="PSUM"))

    # warm the ACT exp spline table behind the initial DMA wait
    warm = big.tile([P, 1], FP32, tag="warm")
    nc.vector.memset(warm, 0.0)
    nc.scalar.activation(warm, warm, mybir.ActivationFunctionType.Exp)

    # input tiles (loaded once; reused across both query passes)
    qts = []
    for s in range(len(sizes)):
        qt = big.tile([P, SQ], F32R, tag=f"qt{s}")
        qts.append(qt)
    kts = big.tile([P, TOT * P], F32R, tag="kts")
    vss = big.tile([P, TOT * P], BF16, tag="vss")
    bias = big.tile([P, TOT], FP32, tag="bias")

    # loads, ordered by first use. DMA descriptors are per-partition
    # (~44ns each regardless of size), so latency-critical transfers are
    # split into partition strips spread across the 16 queues.
    def strips(dst, src, n):
        for i in range(n):
            ps = slice(i * P // n, (i + 1) * P // n)
            nc.sync.dma_start(dst[ps], src[ps])

    seg_offs = []
    o = 0
    for sz in sizes:
        seg_offs.append(o)
        o += sz

    def kt_load(j, n):
        strips(kts[:, j * P : (j + 1) * P], ins["kts"][j], n)

    def vs_load(j, n):
        strips(vss[:, j * P : (j + 1) * P], ins["vss"][j], n)

    def qt_load(s, c, n):
        fs = slice(c * 512, (c + 1) * 512)
        strips(qts[s][:, fs], ins["qts"][s][:, fs], n)

    # segment 0 pass 0 head: fine strips for minimum time-to-first-matmul
    kt_load(seg_offs[0], 4)
    qt_load(0, 0, 4)
    strips(bias, ins["bias"], 2)
    kt_load(seg_offs[0] + 1, 4)
    qt_load(0, 1, 4)
    vs_load(seg_offs[0], 2)
    vs_load(seg_offs[0] + 1, 2)
    for t in range(2, sizes[0]):
        kt_load(seg_offs[0] + t, 2)
        vs_load(seg_offs[0] + t, 2)
    qt_load(0, 2, 2)
    qt_load(0, 3, 2)
    for s in range(1, len(sizes)):
        qt_load(s, 0, 2)
        qt_load(s, 1, 2)
        for t in range(sizes[s]):
            kt_load(seg_offs[s] + t, 2)
            vs_load(seg_offs[s] + t, 2)
        qt_load(s, 2, 2)
        qt_load(s, 3, 2)

    # software pipeline for PV matmuls (PE queue in-order: PV waits on exp)
    pv_q = deque()
    post_q = deque()  # deferred drain work, run one "tile slot" later

    def emit_pv(ot, j, pt, start, stop):
        for h in range(2):
            nc.tensor.matmul(
                ot[:, h * 512 : (h + 1) * 512],
                lhsT=vss[:, j * P : (j + 1) * P],
                rhs=pt[:, h * 512 : (h + 1) * 512],
                start=start,
                stop=stop,
            )

    def flush_one_pv():
        if pv_q:
            emit_pv(*pv_q.popleft())

    seg_offs = []
    o = 0
    for sz in sizes:
        seg_offs.append(o)
        o += sz

    for s, sz in enumerate(sizes):
        for qp in range(NQP):
            qsl = slice(qp * QW, (qp + 1) * QW)
            ot = otp.tile([P, QW], FP32, tag="ot")
            acc = accp.tile([P, QW], BF16, tag="acc")
            for t in range(sz):
                j = seg_offs[s] + t
                if post_q:
                    post_q.popleft()()
                st = stp.tile([P, QW], FP32, tag="st")
                for h in range(2):
                    nc.tensor.matmul(
                        st[:, h * 512 : (h + 1) * 512],
                        lhsT=kts[:, j * P : (j + 1) * P],
                        rhs=qts[s][:, qp * QW + h * 512 : qp * QW + (h + 1) * 512],
                        start=True,
                        stop=True,
                    )
                pt = ptp.tile([P, QW], BF16, tag="pt")
                nc.scalar.activation(
                    pt,
                    st,
                    mybir.ActivationFunctionType.Exp,
                    bias=bias[:, j : j + 1],
                    scale=SCALE,
                )
                if t == 0:
                    nc.vector.tensor_copy(acc, pt)
                else:
                    nc.vector.tensor_add(acc, acc, pt)
                pv_q.append((ot, j, pt, t == 0, t == sz - 1))
                if len(pv_q) > 2:
                    flush_one_pv()

            # drain this pass's outputs once its trailing PVs have flushed
            last = s == len(sizes) - 1 and qp == NQP - 1
            nst = 4 if last else 2

            def tail(s=s, qp=qp, ot=ot, acc=acc, nst=nst):
                strips(outs["dn"][s][qp], acc, nst)
                on = osb.tile([P, QW], BF16, tag="on")
                nc.vector.tensor_copy(on, ot)
                strips(outs["ot"][s][:, qp * QW : (qp + 1) * QW], on, nst)

            # tail must run after the pv_q entries for this pass are emitted;
            # with depth 2, defer by 2 tile slots
            def deferred_tail(t=tail):
                flush_one_pv()
                flush_one_pv()
                t()

            post_q.append(deferred_tail)

    while pv_q:
        flush_one_pv()
    while post_q:
        post_q.popleft()()


_NC_CACHE = {}


def _get_nc(sizes):
    key = tuple(sizes)
    if key in _NC_CACHE:
        return _NC_CACHE[key]
    from contextlib import ExitStack

    S = len(sizes)
    TOT = sum(sizes)
    nc = bacc.Bacc(
        "TRN2",
        target_bir_lowering=False,
        debug=False,
        enable_asserts=False,
        num_devices=NCORES,
    )
    ins = {
        "qts": nc.dram_tensor("qts", [S, D, SQ], F32R, kind="ExternalInput").ap(),
        "kts": nc.dram_tensor("kts", [TOT, D, P], F32R, kind="ExternalInput").ap(),
        "vss": nc.dram_tensor("vss", [TOT, P, D], BF16, kind="ExternalInput").ap(),
        "bias": nc.dram_tensor("bias", [P, TOT], FP32, kind="ExternalInput").ap(),
    }
    outs = {
        "ot": nc.dram_tensor("ot", [S, D, SQ], BF16, kind="ExternalOutput").ap(),
        "dn": nc.dram_tensor("dn", [S, NQP, P, QW], BF16, kind="ExternalOutput").ap(),
    }
    with tile.TileContext(nc) as tc:
        with ExitStack() as ctx:
            _build_kernel(ctx, tc, outs, ins, sizes)
    nc.compile()
    _NC_CACHE[key] = nc
    return nc


LAST_RESULTS = None


def kernel(q, k, v, valid_len):
    q = np.ascontiguousarray(np.asarray(q, dtype=np.float32))
    k = np.ascontiguousarray(np.asarray(k, dtype=np.float32))
    v = np.ascontiguousarray(np.asarray(v, dtype=np.float32))
    vl = np.asarray(valid_len).astype(np.int64)

    import ml_dtypes

    bf16 = ml_dtypes.bfloat16

    sizes, assign = _plan(vl)
    S = len(sizes)
    TOT = sum(sizes)
    seg_offs = []
    o = 0
    for sz in sizes:
        seg_offs.append(o)
        o += sz

    qT = np.swapaxes(q, 1, 2)  # [B, D, SQ]
    kT = np.swapaxes(k, 1, 2)  # [B, D, SK]
    v_bf = v.astype(bf16)

    in_maps = []
    for c in range(NCORES):
        qts = np.zeros((S, D, SQ), dtype=np.float32)
        kts = np.zeros((TOT, D, P), dtype=np.float32)
        vss = np.zeros((TOT, P, D), dtype=bf16)
        bias = np.full((P, TOT), NEG_BIAS, dtype=np.float32)
        for s in range(S):
            ch = assign.get((c, s))
            if ch is None:
                continue
            b, t0, n = ch
            qts[s] = qT[b]
            for t in range(n):
                j = seg_offs[s] + t
                ks = (t0 + t) * P
                kts[j] = kT[b][:, ks : ks + P]
                vss[j] = v_bf[b][ks : ks + P]
                nvalid = int(min(P, max(0, vl[b] - ks)))
                bias[:nvalid, j] = 0.0
        in_maps.append({"qts": qts, "kts": kts, "vss": vss, "bias": bias})

    nc = _get_nc(sizes)
    tr = int(os.environ.get("KERNEL_TRACE", "0"))
    res = run_bass_kernel_spmd(
        nc,
        in_maps,
        core_ids=list(range(NCORES)),
        trace=tr > 0,
        trace_cores=(list(range(NCORES)) if tr == 2 else [0]) if tr else None,
    )
    global LAST_RESULTS
    LAST_RESULTS = res

    O_acc = np.zeros((B, D, SQ), dtype=np.float32)
    den = np.zeros((B, SQ), dtype=np.float32)
    for c in range(NCORES):
        r = res.results[c]
        ot = np.asarray(r["ot"], dtype=np.float32)  # [S, D, SQ]
        dn = np.asarray(r["dn"], dtype=np.float32)  # [S, NQP, P, QW]
        for s in range(S):
            ch = assign.get((c, s))
            if ch is None:
                continue
            b, t0, n = ch
            O_acc[b] += ot[s]
            for qp in range(NQP):
                den[b][qp * QW : (qp + 1) * QW] += dn[s][qp].sum(axis=0)

    out = np.empty((B, SQ, D), dtype=np.float32)
    for b in range(B):
        if vl[b] <= 0:
            out[b] = v[b].mean(axis=0, keepdims=True)
        else:
            out[b] = (O_acc[b] / np.maximum(den[b][None, :], 1e-30)).T
    return out.astype(np.float32)


# revision 9
# speedup vs baseline: 1.5658x; 1.5658x over previous
"""Masked dot-product attention on 8 Trainium2 NeuronCores.

Problem: q,k,v [16, 2048, 128] fp32, valid_len [16] int -> out [16, 2048, 128].
out[b] = softmax(mask(q[b] @ k[b].T / sqrt(128), valid_len[b])) @ v[b]

Strategy (v2, ~2-3x over the batch-parallel flash baseline):
  - Keys beyond valid_len contribute exp(-inf)=0 exactly, so fully-masked
    128-key tiles can be skipped. For the graded input only ~127 of 256
    (batch, key-tile) pairs have any valid keys.
  - The work unit is a (batch, key-tile) pair. A single SPMD program runs
    S fixed-size "segments" per core (sizes compile-time, e.g. (10,5,3));
    the host binds each (core, segment) slot to any batch + tile range and
    sums the partial results: each segment emits an UNNORMALIZED O^T
    [d,2048] plus per-query exp-sums; host adds partials per batch and
    divides. Unused slot capacity is padded with zero tiles + -inf bias.
  - Mask is applied as the ACT per-partition bias: st layout is
    [key partition, query free], so a [128,1] bias of 0/-30000 per key
    tile masks invalid keys inside the exp instruction. No mask matmul
    stream (1/3 of baseline PE work), no zeroed-V copy, no mb tensor.
  - Per segment, per 1024-query pass: score matmul kt_t.T @ qt -> st
    (PSUM, f32r), ACT exp(st*scale+bias) -> pt (SBUF, bf16), PV matmul
    vs_t.T @ pt accumulating OT in PSUM across the tile loop, DVE
    acc += pt (bf16) accumulates the softmax denominators; acc tiles are
    DMA'd out raw and partition-summed on host.
  - OT is drained PSUM->SBUF (cast bf16) on the otherwise-idle GPSIMD
    engine, then DMA'd. PE runs only the two irreducible matmul streams.
"""

import os
from collections import deque

import numpy as np

import concourse.tile as tile
from concourse import bacc, mybir
from concourse.bass_utils import run_bass_kernel_spmd

B, SQ, SK, D = 16, 2048, 2048, 128
NCORES = 8
P = 128
QW = 1024  # query pass width (PSUM: OT [128,1024] fp32 = 2 banks)
NQP = SQ // QW  # query passes
SCALE = 1.0 / float(np.sqrt(D))
NEG_BIAS = -30000.0

FP32 = mybir.dt.float32
F32R = mybir.dt.float32r
BF16 = mybir.dt.bfloat16

# ---------------------------------------------------------------------------
# packing: cover per-batch tile counts with 8 copies of each segment size
# ---------------------------------------------------------------------------

# candidate segment-size tuples, tried in order (first feasible wins)
_CANDIDATES = [
    (10, 5, 3),
    (10, 6, 3),
    (11, 5, 3),
    (11, 6, 4),
    (12, 6, 4),
    (13, 7, 4),
    (14, 7, 5),
    (16, 8, 8),  # always feasible for 16 batches of <=16 tiles
]


def _try_pack(nt, sizes, rng):
    """Try to cover tile counts nt (list of (batch, count)) with 8 slots of
    each size in `sizes`. Returns list of chunks (batch, tile_start, n, size)
    or None."""
    slots = []
    for sz in sizes:
        slots += [sz] * 8
    slots.sort(reverse=True)
    order = sorted(range(len(nt)), key=lambda i: -nt[i][1])
    if rng is not None:
        order = list(order)
        rng.shuffle(order)
    avail = list(slots)
    chunks = []
    for i in order:
        b, cnt = nt[i]
        t0 = 0
        rem = cnt
        while rem > 0:
            # if some slot can finish the batch, take the smallest such;
            # otherwise take the largest slot and continue
            ge = [s for s in avail if s >= rem]
            if ge:
                s = min(ge)
                if rng is not None and len(ge) > 1 and rng.rand() < 0.3:
                    s = rng.choice(ge)
            else:
                if not avail:
                    return None
                s = max(avail)
            avail.remove(s)
            take = min(s, rem)
            chunks.append((b, t0, take, s))
            t0 += take
            rem -= take
    return chunks


def _plan(valid_len):
    """Choose segment sizes + assignment of chunks to (core, seg) slots."""
    nt = []
    for b in range(B):
        c = int(min(SK, max(0, int(valid_len[b]))))
        c = (c + P - 1) // P
        if c > 0:
            nt.append((b, c))
    rng = np.random.RandomState(0)
    for sizes in _CANDIDATES:
        if sum(sizes) * 8 < sum(c for _, c in nt):
            continue
        for trial in range(64):
            chunks = _try_pack(nt, sizes, None if trial == 0 else rng)
            if chunks is not None:
                # map chunks to (core, seg) slots: slot list per size
                free = {}
                for s_idx, sz in enumerate(sizes):
                    free.setdefault(sz, [])
                    free[sz] += [(c, s_idx) for c in range(NCORES)]
                assign = {}  # (core, seg) -> (batch, t0, n)
                ok = True
                for b, t0, n, sz in chunks:
                    if not free[sz]:
                        ok = False
                        break
                    core, s_idx = free[sz].pop()
                    assign[(core, s_idx)] = (b, t0, n)
                if ok:
                    return sizes, assign
    raise RuntimeError("packing failed")


# ---------------------------------------------------------------------------
# device kernel (one per segment-size tuple, cached)
# ---------------------------------------------------------------------------


def _build_kernel(ctx, tc, outs, ins, sizes):
    nc = tc.nc
    TOT = sum(sizes)
    big = ctx.enter_context(tc.tile_pool(name="big", bufs=1))
    ptp = ctx.enter_context(tc.tile_pool(name="ptp", bufs=4))
    accp = ctx.enter_context(tc.tile_pool(name="accp", bufs=2))
    osb = ctx.enter_context(tc.tile_pool(name="osb", bufs=2))
    stp = ctx.enter_context(tc.tile_pool(name="stp", bufs=2, space="PSUM"))
    otp = ctx.enter_context(tc.tile_pool(name="otp", bufs=2, space="PSUM"))

    # warm the ACT exp spline table behind the initial DMA wait
    warm = big.tile([P, 1], FP32, tag="warm")
    nc.vector.memset(warm, 0.0)
    nc.scalar.activation(warm, warm, mybir.ActivationFunctionType.Exp)

    # input tiles (loaded once; reused across both query passes)
    qts = []
    for s in range(len(sizes)):
        qt = big.tile([P, SQ], F32R, tag=f"qt{s}")
        qts.append(qt)
    kts = big.tile([P, TOT * P], F32R, tag="kts")
    vss = big.tile([P, TOT * P], BF16, tag="vss")
    bias = big.tile([P, TOT], FP32, tag="bias")

    # loads, ordered by first use. Each dma_start costs ~600ns of serialized
    # dispatch on its issuing sequencer, and one instruction's descriptors go
    # to a single DMA queue (~25-90ns per partition row). So: few critical
    # first-tile transfers (partition-striped for parallel queues) go on the
    # sync queue; all remaining loads dispatch from the idle GpSimd
    # sequencer, one instruction per tile/chunk, in first-use order.
    def strips(dst, src, n, eng=None):
        eng = eng or nc.sync
        for i in range(n):
            ps = slice(i * P // n, (i + 1) * P // n)
            eng.dma_start(dst[ps], src[ps])

    seg_offs = []
    o = 0
    for sz in sizes:
        seg_offs.append(o)
        o += sz

    def kt_load(j, eng, n=1):
        strips(kts[:, j * P : (j + 1) * P], ins["kts"][j], n, eng)

    def vs_load(j, eng, n=1):
        strips(vss[:, j * P : (j + 1) * P], ins["vss"][j], n, eng)

    def qt_load(s, c, eng, n=1):
        fs = slice(c * 512, (c + 1) * 512)
        strips(qts[s][:, fs], ins["qts"][s][:, fs], n, eng)

    # critical head: segment 0 tile 0/1 operands, striped across queues
    kt_load(seg_offs[0], nc.sync, 2)
    qt_load(0, 0, nc.sync, 2)
    nc.sync.dma_start(bias, ins["bias"])
    kt_load(seg_offs[0] + 1, nc.sync)
    qt_load(0, 1, nc.sync)
    vs_load(seg_offs[0], nc.sync)
    vs_load(seg_offs[0] + 1, nc.sync)
    # bulk: GpSimd sequencer, first-use order
    g = nc.gpsimd
    for t in range(2, sizes[0]):
        kt_load(seg_offs[0] + t, g)
        vs_load(seg_offs[0] + t, g)
    qt_load(0, 2, g)
    qt_load(0, 3, g)
    for s in range(1, len(sizes)):
        qt_load(s, 0, g)
        qt_load(s, 1, g)
        for t in range(sizes[s]):
            kt_load(seg_offs[s] + t, g)
            vs_load(seg_offs[s] + t, g)
        qt_load(s, 2, g)
        qt_load(s, 3, g)

    # software pipeline for PV matmuls (PE queue in-order: PV waits on exp)
    pv_q = deque()
    post_q = deque()  # deferred drain work, run one "tile slot" later

    def emit_pv(ot, j, pt, start, stop):
        for h in range(2):
            nc.tensor.matmul(
                ot[:, h * 512 : (h + 1) * 512],
                lhsT=vss[:, j * P : (j + 1) * P],
                rhs=pt[:, h * 512 : (h + 1) * 512],
                start=start,
                stop=stop,
            )

    def flush_one_pv():
        if pv_q:
            emit_pv(*pv_q.popleft())

    seg_offs = []
    o = 0
    for sz in sizes:
        seg_offs.append(o)
        o += sz

    for s, sz in enumerate(sizes):
        for qp in range(NQP):
            qsl = slice(qp * QW, (qp + 1) * QW)
            ot = otp.tile([P, QW], FP32, tag="ot")
            acc = accp.tile([P, QW], BF16, tag="acc")
            for t in range(sz):
                j = seg_offs[s] + t
                if post_q:
                    post_q.popleft()()
                st = stp.tile([P, QW], FP32, tag="st")
                for h in range(2):
                    nc.tensor.matmul(
                        st[:, h * 512 : (h + 1) * 512],
                        lhsT=kts[:, j * P : (j + 1) * P],
                        rhs=qts[s][:, qp * QW + h * 512 : qp * QW + (h + 1) * 512],
                        start=True,
                        stop=True,
                    )
                pt = ptp.tile([P, QW], BF16, tag="pt")
                nc.scalar.activation(
                    pt,
                    st,
                    mybir.ActivationFunctionType.Exp,
                    bias=bias[:, j : j + 1],
                    scale=SCALE,
                )
                if t == 0:
                    nc.vector.tensor_copy(acc, pt)
                else:
                    nc.vector.tensor_add(acc, acc, pt)
                pv_q.append((ot, j, pt, t == 0, t == sz - 1))
                if len(pv_q) > 2:
                    flush_one_pv()

            # drain this pass's outputs once its trailing PVs have flushed
            last = s == len(sizes) - 1 and qp == NQP - 1
            nst = 4 if last else 2

            def tail(s=s, qp=qp, ot=ot, acc=acc, nst=nst):
                strips(outs["dn"][s][qp], acc, nst)
                on = osb.tile([P, QW], BF16, tag="on")
                nc.vector.tensor_copy(on, ot)
                strips(outs["ot"][s][:, qp * QW : (qp + 1) * QW], on, nst)

            # tail must run after the pv_q entries for this pass are emitted;
            # with depth 2, defer by 2 tile slots
            def deferred_tail(t=tail):
                flush_one_pv()
                flush_one_pv()
                t()

            post_q.append(deferred_tail)

    while pv_q:
        flush_one_pv()
    while post_q:
        post_q.popleft()()


_NC_CACHE = {}


def _get_nc(sizes):
    key = tuple(sizes)
    if key in _NC_CACHE:
        return _NC_CACHE[key]
    from contextlib import ExitStack

    S = len(sizes)
    TOT = sum(sizes)
    nc = bacc.Bacc(
        "TRN2",
        target_bir_lowering=False,
        debug=False,
        enable_asserts=False,
        num_devices=NCORES,
    )
    ins = {
        "qts": nc.dram_tensor("qts", [S, D, SQ], F32R, kind="ExternalInput").ap(),
        "kts": nc.dram_tensor("kts", [TOT, D, P], F32R, kind="ExternalInput").ap(),
        "vss": nc.dram_tensor("vss", [TOT, P, D], BF16, kind="ExternalInput").ap(),
        "bias": nc.dram_tensor("bias", [P, TOT], FP32, kind="ExternalInput").ap(),
    }
    outs = {
        "ot": nc.dram_tensor("ot", [S, D, SQ], BF16, kind="ExternalOutput").ap(),
        "dn": nc.dram_tensor("dn", [S, NQP, P, QW], BF16, kind="ExternalOutput").ap(),
    }
    with tile.TileContext(nc) as tc:
        with ExitStack() as ctx:
            _build_kernel(ctx, tc, outs, ins, sizes)
    nc.compile()
    _NC_CACHE[key] = nc
    return nc


LAST_RESULTS = None


def kernel(q, k, v, valid_len):
    q = np.ascontiguousarray(np.asarray(q, dtype=np.float32))
    k = np.ascontiguousarray(np.asarray(k, dtype=np.float32))
    v = np.ascontiguousarray(np.asarray(v, dtype=np.float32))
    vl = np.asarray(valid_len).astype(np.int64)

    import ml_dtypes

    bf16 = ml_dtypes.bfloat16

    sizes, assign = _plan(vl)
    S = len(sizes)
    TOT = sum(sizes)
    seg_offs = []
    o = 0
    for sz in sizes:
        seg_offs.append(o)
        o += sz

    qT = np.swapaxes(q, 1, 2)  # [B, D, SQ]
    kT = np.swapaxes(k, 1, 2)  # [B, D, SK]
    v_bf = v.astype(bf16)

    in_maps = []
    for c in range(NCORES):
        qts = np.zeros((S, D, SQ), dtype=np.float32)
        kts = np.zeros((TOT, D, P), dtype=np.float32)
        vss = np.zeros((TOT, P, D), dtype=bf16)
        bias = np.full((P, TOT), NEG_BIAS, dtype=np.float32)
        for s in range(S):
            ch = assign.get((c, s))
            if ch is None:
                continue
            b, t0, n = ch
            qts[s] = qT[b]
            for t in range(n):
                j = seg_offs[s] + t
                ks = (t0 + t) * P
                kts[j] = kT[b][:, ks : ks + P]
                vss[j] = v_bf[b][ks : ks + P]
                nvalid = int(min(P, max(0, vl[b] - ks)))
                bias[:nvalid, j] = 0.0
        in_maps.append({"qts": qts, "kts": kts, "vss": vss, "bias": bias})

    nc = _get_nc(sizes)
    tr = int(os.environ.get("KERNEL_TRACE", "0"))
    res = run_bass_kernel_spmd(
        nc,
        in_maps,
        core_ids=list(range(NCORES)),
        trace=tr > 0,
        trace_cores=(list(range(NCORES)) if tr == 2 else [0]) if tr else None,
    )
    global LAST_RESULTS
    LAST_RESULTS = res

    O_acc = np.zeros((B, D, SQ), dtype=np.float32)
    den = np.zeros((B, SQ), dtype=np.float32)
    for c in range(NCORES):
        r = res.results[c]
        ot = np.asarray(r["ot"], dtype=np.float32)  # [S, D, SQ]
        dn = np.asarray(r["dn"], dtype=np.float32)  # [S, NQP, P, QW]
        for s in range(S):
            ch = assign.get((c, s))
            if ch is None:
                continue
            b, t0, n = ch
            O_acc[b] += ot[s]
            for qp in range(NQP):
                den[b][qp * QW : (qp + 1) * QW] += dn[s][qp].sum(axis=0)

    out = np.empty((B, SQ, D), dtype=np.float32)
    for b in range(B):
        if vl[b] <= 0:
            out[b] = v[b].mean(axis=0, keepdims=True)
        else:
            out[b] = (O_acc[b] / np.maximum(den[b][None, :], 1e-30)).T
    return out.astype(np.float32)
